# revision 2
# baseline (speedup 1.0000x reference)
"""AttentionLSTM kernel for 8 trn2 NeuronCores.

Strategy (per sharding hint): pure data-parallel over the batch dim N=128.
Each of the 8 cores runs the T=256 LSTM-with-attention recurrence on its
own 16 batch rows; weights are replicated. Every batch element's recurrence
is independent, so this sharding needs zero per-step collectives - the only
distribution cost is the initial scatter and the final gather.

The recurrence is executed in 16-step chunks (one jitted program per chunk
length) because the neuron compiler's per-program instruction-count limit
rejects the fully-unrolled 256-step scan. (h, c) carry between chunks stays
on-device; A and the weights are staged on-device once.

Hardcoded problem shapes: x (128, 256, 1024) f32, A (128, 1024, 14, 14) f32,
Wx/Wh/Wattn (1024, 4096) f32, b (4096,) f32 -> out (128, 256, 1024) f32.
"""

import numpy as np

N, T, D, H, DA = 128, 256, 1024, 1024, 14
NCORES = 8
NSHARD = N // NCORES
CHUNK = 16


def _steps(h, c, xch, A_flat, Wx, Wh, Wattn, b):
    """Run xch.shape[1] LSTM+attention steps. h/c (n, H), xch (n, t, D)."""
    import jax
    import jax.numpy as jnp

    scale = jnp.float32(1.0 / np.sqrt(A_flat.shape[1]))

    def step(carry, xt):
        h, c = carry
        scores = jnp.einsum('nh,nhp->np', h, A_flat) * scale
        w = jax.nn.softmax(scores, axis=-1)
        attn = jnp.einsum('nhp,np->nh', A_flat, w)
        a = xt @ Wx + h @ Wh + attn @ Wattn + b
        ai, af, ao, ag = jnp.split(a, 4, axis=1)
        i = jax.nn.sigmoid(ai)
        f = jax.nn.sigmoid(af)
        o = jax.nn.sigmoid(ao)
        g = jnp.tanh(ag)
        c_next = f * c + i * g
        h_next = o * jnp.tanh(c_next)
        return (h_next, c_next), h_next

    (h, c), hs = jax.lax.scan(step, (h, c), jnp.swapaxes(xch, 0, 1))
    return h, c, jnp.swapaxes(hs, 0, 1)


def _run_sharded(x, A, Wx, Wh, Wattn, b, devices):
    import jax
    import jax.numpy as jnp

    A_sh = jax.device_put_sharded(
        [A[i * NSHARD:(i + 1) * NSHARD].reshape(NSHARD, H, DA * DA)
         for i in range(NCORES)], devices)
    Wx_r = jax.device_put_replicated(Wx, devices)
    Wh_r = jax.device_put_replicated(Wh, devices)
    Wa_r = jax.device_put_replicated(Wattn, devices)
    b_r = jax.device_put_replicated(b, devices)

    init = jax.pmap(lambda Af: (jnp.mean(Af, axis=-1),) * 2, devices=devices)
    h, c = init(A_sh)

    fch = jax.pmap(_steps, in_axes=(0,) * 8, devices=devices)

    out = np.empty((NCORES, NSHARD, T, H), np.float32)
    for t0 in range(0, T, CHUNK):
        x_sh = jax.device_put_sharded(
            [np.ascontiguousarray(x[i * NSHARD:(i + 1) * NSHARD, t0:t0 + CHUNK])
             for i in range(NCORES)], devices)
        h, c, hs = fch(h, c, x_sh, A_sh, Wx_r, Wh_r, Wa_r, b_r)
        out[:, :, t0:t0 + CHUNK] = np.asarray(hs)
    return out.reshape(N, T, H)


def kernel(x, A, Wx, Wh, Wattn, b):
    x = np.asarray(x, dtype=np.float32)
    A = np.asarray(A, dtype=np.float32)
    Wx = np.asarray(Wx, dtype=np.float32)
    Wh = np.asarray(Wh, dtype=np.float32)
    Wattn = np.asarray(Wattn, dtype=np.float32)
    b = np.asarray(b, dtype=np.float32)

    import jax

    # Preferred path: the 8 axon-tunneled trn2 NeuronCores.
    try:
        devs = [d for d in jax.devices() if d.platform != 'cpu'][:NCORES]
        if len(devs) == NCORES:
            return _run_sharded(x, A, Wx, Wh, Wattn, b, devs)
    except Exception:
        pass

    # Fallback: CPU execution, batch-chunked to bound memory.
    import jax.numpy as jnp

    cpu = jax.devices('cpu')[0]
    with jax.default_device(cpu):
        fn = jax.jit(_steps)
        out = np.empty((N, T, H), dtype=np.float32)
        for s in range(0, N, NSHARD):
            A_flat = jnp.asarray(A[s:s + NSHARD].reshape(NSHARD, H, DA * DA))
            h = jnp.mean(A_flat, axis=-1)
            c = h
            for t0 in range(0, T, CHUNK):
                h, c, hs = fn(h, c, x[s:s + NSHARD, t0:t0 + CHUNK],
                              A_flat, Wx, Wh, Wattn, b)
                out[s:s + NSHARD, t0:t0 + CHUNK] = np.asarray(hs)
        return out
